# revision 19
# baseline (speedup 1.0000x reference)
"""Contrastive-loss kernel for 8 TRN2 NeuronCores (Bass/Tile, SPMD).

Math (reference, margin=1):
    d_ij = |x_i|^2 + |x_j|^2 - 2 x_i.x_j            (clamped >= 0)
    pos  = sum_{i!=j, same class} d_ij
    neg  = sum_{i!=j, diff class} relu(1 - sqrt(d_ij))^2
    loss = (pos + neg) / (2 n (n-1))

Algorithm:
  * pos via the exact class-sum identity
        sum_{i,j in c} d_ij = 2 n_c S2_c - 2 |S1_c|^2
    (diagonal terms are 0, so the i!=j sum equals the full sum).
    S1_c = sum_{i in c} x_i is computed ON DEVICE with small bf16
    matmuls (onehot^T @ X per 128-row chunk, accumulated in PSUM);
    S2_c / n_c are O(N) host reductions.
  * neg is zero iff every different-class pair has d >= 1.  The device
    certifies this: it computes the full Gram matrix G = X X^T (fp8
    DoubleRow matmuls, block-pair tiled, each unordered block-pair
    once) and reduces each [128,512] tile to a max (or a
    relu-threshold detector on the Scalar engine).  Host check:
        min d >= min_row |x_i|^2 + min_col |x_j|^2 - 2 (maxG + FP8ERR)
    with exact host-side min-norms and a rigorous fp8 error bound.
    If any tile fails the bound (never, for real data), the host
    recomputes neg exactly.
  * Self block-pairs (a==b) would have the max dominated by the
    diagonal G_ii = |x_i|^2, so they get one extra bf16 matmul adding
    -lam^2 * same_class(i,j) (lam=256) which pushes all same-class
    entries (incl. the diagonal) to ~-65536, leaving the max over
    different-class pairs only.
  * Block-pair coverage: the 136 unordered pairs of 16 row-blocks are
    oriented by the circulant tournament (edge {a,b} belongs to a iff
    (b-a) mod 16 <= 7, diameter edges to the low block).  Core k owns
    blocks {k, 15-k}: exactly 9 pairs for block k (partners k..k+8
    mod 16) and 8 for block 15-k -- a fixed SPMD template, with the
    self pairs at t=0 and t=9.
  * Reductions are split DVE (max over free axis) / Scalar (relu
    detector, threshold 250) so both stay under the TensorE pipeline
    (GpSimd cannot read PSUM on TRN2).  A few dummy bf16 matmuls at
    t=0 keep the PE busy during the first DMAs so the HAM clock-gate
    warms up early.
"""

import os

import numpy as np
import ml_dtypes

N, C, NCLS = 8192, 512, 100
NB, BS = 16, 512          # row blocks
NPAIR = 17                # block-pairs per core (t=0 / t=9 are self pairs)
NDU = NPAIR * 2           # reduce granularity: [128, 1024] double-tiles
LAM = 256.0
MARGIN = 1.0
THRESH = 250.0            # scalar-engine relu detector threshold on G
# rigorous |G_fp8 - G_true| bound: per-term rel err <= 2*2^-4 + 2^-8,
# sum_k |x_ik||x_jk| <= |x_i||x_j| <= max_sq (~660 for randn; the host
# check recomputes this bound from the actual data)
FP8_RELERR = 0.13

FP8 = ml_dtypes.float8_e4m3
BF16 = ml_dtypes.bfloat16

_CACHE: dict = {}


def _red_engine(du: int) -> str:
    # V=DVE max-reduce, A=Scalar relu detector (GpSimd cannot read
    # PSUM on TRN2).  19/34 to DVE, 15/34 to Scalar; the scalar queue
    # first drains its DMA issues and the act-table load, so the first
    # four double-units go to DVE.
    if du < 4:
        return "V"
    return "V" if du % 2 == 0 else "A"


def _build_bass():
    import concourse.bacc as bacc
    import concourse.mybir as mybir
    import concourse.tile as tile

    nc = bacc.Bacc(
        "TRN2",
        target_bir_lowering=False,
        debug=False,
        enable_asserts=False,
        num_devices=8,
    )
    ablk_d = nc.dram_tensor(
        "ablk", [2, 128, 2048], mybir.dt.uint8, kind="ExternalInput"
    ).ap()
    bblk_d = nc.dram_tensor(
        "bblk", [NPAIR, 128, 2048], mybir.dt.uint8, kind="ExternalInput"
    ).ap()
    # X in [row-in-chunk, chunk, chan] layout for the S1 contraction (bf16)
    xga_d = nc.dram_tensor(
        "xga", [2, 128, 4096], mybir.dt.uint8, kind="ExternalInput"
    ).ap()
    # onehot (1.0) of the rows, [row-in-chunk, chunk, class] (bf16)
    oha_d = nc.dram_tensor(
        "oha", [2, 128, 800], mybir.dt.uint8, kind="ExternalInput"
    ).ap()
    # +lam*onehot / -lam*onehot per self block, [class, row] (bf16)
    ohp_d = nc.dram_tensor(
        "ohp", [2, 100, 1024], mybir.dt.uint8, kind="ExternalInput"
    ).ap()
    ohm_d = nc.dram_tensor(
        "ohm", [2, 100, 1024], mybir.dt.uint8, kind="ExternalInput"
    ).ap()
    mx_d = nc.dram_tensor(
        "mx", [128, NDU], mybir.dt.float32, kind="ExternalOutput"
    ).ap()
    acc_d = nc.dram_tensor(
        "acc", [128, NDU], mybir.dt.float32, kind="ExternalOutput"
    ).ap()
    s1_d = nc.dram_tensor(
        "s1", [100, 512], mybir.dt.float32, kind="ExternalOutput"
    ).ap()

    DR = mybir.MatmulPerfMode.DoubleRow

    with tile.TileContext(nc) as tc:
        with (
            tc.tile_pool(name="io", bufs=1) as iop,
            tc.tile_pool(name="psp", bufs=3, space="PSUM") as psp,
            tc.tile_pool(name="ps1", bufs=1, space="PSUM") as ps1p,
            tc.tile_pool(name="psw", bufs=1, space="PSUM") as pswp,
        ):
            mx = iop.tile([128, NDU], mybir.dt.float32)
            nc.gpsimd.memset(mx[:], -3.0e38)
            acc = iop.tile([128, NDU], mybir.dt.float32)
            nc.gpsimd.memset(acc[:], 0.0)
            scr = iop.tile([128, 1024], mybir.dt.bfloat16)
            thr = iop.tile([128, 1], mybir.dt.float32)
            nc.gpsimd.memset(thr[:], -THRESH)

            # --- warmup: keep PE busy while the first blocks DMA in ---
            warm = iop.tile([128, 512], mybir.dt.bfloat16)
            nc.gpsimd.memset(warm[:], 0.0)
            psw = pswp.tile([128, 512], mybir.dt.float32)
            for w in range(2):
                nc.tensor.matmul(
                    psw[:], warm[:, 0:128], warm[:],
                    start=(w == 0), stop=(w == 1),
                )

            # --- input DMAs (all issued upfront; strict-FIFO engine
            # queues mean a consumer emitted before its DMA issue on the
            # same queue would deadlock).  Everything rides the sync
            # queue except the S1 operands (GpSimd SWDGE) -- the scalar
            # queue stays free for the detector activations.
            at = iop.tile([128, 2, 2048], mybir.dt.uint8)
            bt = iop.tile([128, NPAIR, 2048], mybir.dt.uint8)
            # critical path (pair blocks) on the sync HWDGE queue
            nc.sync.dma_start(at[:, 0, :], ablk_d[0])
            nc.sync.dma_start(bt[:, 0:1, :], bblk_d[0:1])
            nc.sync.dma_start(at[:, 1, :], ablk_d[1])
            bchunks = [(1, 2), (2, 4), (4, 7), (7, 10), (10, 13), (13, 17)]
            for lo, hi in bchunks:
                nc.sync.dma_start(bt[:, lo:hi, :], bblk_d[lo:hi])
            # masks + S1 operands on the scalar HWDGE queue (done well
            # before the first detector activation at du=5)
            ohp, ohm = [], []
            for i in range(2):
                p = iop.tile([100, 1024], mybir.dt.uint8)
                nc.scalar.dma_start(p[:], ohp_d[i])
                ohp.append(p)
                m = iop.tile([100, 1024], mybir.dt.uint8)
                nc.scalar.dma_start(m[:], ohm_d[i])
                ohm.append(m)
            xga, oha = [], []
            for i in range(2):
                x = iop.tile([128, 4096], mybir.dt.uint8)
                nc.scalar.dma_start(x[:], xga_d[i])
                xga.append(x)
                o = iop.tile([128, 800], mybir.dt.uint8)
                nc.scalar.dma_start(o[:], oha_d[i])
                oha.append(o)

            a8 = at[:].bitcast(mybir.dt.float8e4).rearrange(
                "p t (c i n) -> p t c i n", c=2, i=2
            )
            b8 = bt[:].bitcast(mybir.dt.float8e4).rearrange(
                "p t (c i n) -> p t c i n", c=2, i=2
            )

            def pair(t):
                ai = 0 if t < 9 else 1
                self_pair = t in (0, 9)
                for half in range(2):
                    du = t * 2 + half
                    ps = psp.tile([128, 1024], mybir.dt.float32)
                    for sub in range(2):
                        rt = half * 2 + sub
                        out = ps[:, sub * 512 : (sub + 1) * 512]
                        sl = slice(rt * 128, (rt + 1) * 128)
                        nc.tensor.matmul(
                            out, a8[:, ai, 0, :, sl], b8[:, t, 0, :, :],
                            start=True, stop=False, perf_mode=DR,
                        )
                        nc.tensor.matmul(
                            out, a8[:, ai, 1, :, sl], b8[:, t, 1, :, :],
                            start=False, stop=not self_pair, perf_mode=DR,
                        )
                        if self_pair:
                            pv = ohp[ai][:].bitcast(mybir.dt.bfloat16)
                            mv = ohm[ai][:].bitcast(mybir.dt.bfloat16)
                            nc.tensor.matmul(
                                out, pv[:, sl], mv[:],
                                start=False, stop=True,
                            )
                    eng = _red_engine(du)
                    if eng == "V":
                        nc.vector.tensor_reduce(
                            mx[:, du : du + 1], ps[:],
                            axis=mybir.AxisListType.X, op=mybir.AluOpType.max,
                        )
                    else:
                        nc.scalar.activation(
                            scr[:], ps[:],
                            mybir.ActivationFunctionType.Relu,
                            bias=thr[:], scale=1.0,
                            accum_out=acc[:, du : du + 1],
                        )

            pair(0)
            pair(1)

            # --- S1 partial class sums over this core's two A blocks ---
            pss1 = ps1p.tile([128, 512], mybir.dt.float32)
            for i in range(2):
                xv = xga[i][:].bitcast(mybir.dt.bfloat16).rearrange(
                    "p (h n) -> p h n", h=4
                )
                ov = oha[i][:].bitcast(mybir.dt.bfloat16).rearrange(
                    "p (h m) -> p h m", h=4
                )
                for h in range(4):
                    nc.tensor.matmul(
                        pss1[0:100, :], ov[:, h, :], xv[:, h, :],
                        start=(i == 0 and h == 0), stop=(i == 1 and h == 3),
                    )

            for t in range(2, NPAIR):
                pair(t)

            s1sb = iop.tile([128, 512], mybir.dt.float32)
            nc.vector.tensor_copy(s1sb[0:100, :], pss1[0:100, :])

            nc.sync.dma_start(mx_d[:], mx[:])
            nc.sync.dma_start(acc_d[:], acc[:])
            nc.sync.dma_start(s1_d[:], s1sb[0:100, :])

    nc.compile()
    return nc


def _pair_lists():
    """Per-core block-pair template from the circulant tournament."""
    cores = []
    for k in range(8):
        a0, a1 = k, 15 - k
        pairs = [(a0, (a0 + t) % 16) for t in range(9)]
        pairs += [(a1, (a1 + t) % 16) for t in range(8)]
        assert len(pairs) == NPAIR
        cores.append(pairs)
    # every unordered pair covered exactly once
    seen = set()
    for pairs in cores:
        for a, b in pairs:
            key = (min(a, b), max(a, b))
            assert key not in seen
            seen.add(key)
    assert len(seen) == 136
    return cores


def _prep(features: np.ndarray, target: np.ndarray):
    f = np.ascontiguousarray(features, np.float32)
    tg = np.asarray(target).astype(np.int64)

    # fp8 feature blocks, DoubleRow layout: chan = 256c + 128i + p, col = row
    X8 = f.astype(FP8).reshape(NB, BS, 2, 2, 128)      # [blk, m, c, i, p]
    F8 = np.ascontiguousarray(X8.transpose(0, 4, 2, 3, 1))  # [blk, p, c, i, m]
    F8 = F8.reshape(NB, 128, 2048).view(np.uint8)

    # bf16 X in [row-in-chunk(p), chunk, chan] layout per block
    XG = np.ascontiguousarray(
        f.reshape(NB, 4, 128, C).transpose(0, 2, 1, 3).astype(BF16)
    )  # [blk, 128, 4, 512] bf16
    XG = XG.view(np.uint8).reshape(NB, 128, 4096)

    # onehot(1.0) of rows, [row-in-chunk, chunk, class]
    OH = np.zeros((N, NCLS), np.float32)
    OH[np.arange(N), tg] = 1.0
    OHA = np.ascontiguousarray(
        OH.reshape(NB, 4, 128, NCLS).transpose(0, 2, 1, 3).astype(BF16)
    )
    OHA = OHA.view(np.uint8).reshape(NB, 128, 800)

    # +-lam*onehot, [class, row] per block
    OHT = np.zeros((NB, NCLS, BS), np.float32)
    for blk in range(NB):
        OHT[blk, tg[blk * BS : (blk + 1) * BS], np.arange(BS)] = LAM
    OHP = np.ascontiguousarray(OHT.astype(BF16)).view(np.uint8).reshape(
        NB, NCLS, 1024
    )
    OHM = np.ascontiguousarray((-OHT).astype(BF16)).view(np.uint8).reshape(
        NB, NCLS, 1024
    )
    return F8, XG, OHA, OHP, OHM


def _make_in_maps(features: np.ndarray, target: np.ndarray):
    F8, XG, OHA, OHP, OHM = _prep(features, target)
    in_maps = []
    for k, pairs in enumerate(_pair_lists()):
        bi = [b for _, b in pairs]
        sb = [k, 15 - k]  # this core's A blocks
        in_maps.append(
            {
                "ablk": np.ascontiguousarray(F8[sb]),
                "bblk": np.ascontiguousarray(F8[bi]),
                "xga": np.ascontiguousarray(XG[sb]),
                "oha": np.ascontiguousarray(OHA[sb]),
                "ohp": np.ascontiguousarray(OHP[sb]),
                "ohm": np.ascontiguousarray(OHM[sb]),
            }
        )
    return in_maps


def _host_neg_term(features: np.ndarray, target: np.ndarray) -> float:
    """Exact recompute of the negative (hinge) term; only runs if the
    on-device distance certificate fails (never, for real data)."""
    f = np.asarray(features, np.float32)
    sq = (f * f).sum(1)
    d = sq[:, None] + sq[None, :] - 2.0 * (f @ f.T)
    d = np.maximum(d, 0.0)
    tg = np.asarray(target)
    same = tg[:, None] == tg[None, :]
    eye = np.eye(N, dtype=bool)
    neg_mask = (~same) & (~eye)
    tmp = np.where(d > 0, MARGIN - np.sqrt(np.where(d > 0, d, 1.0)), MARGIN)
    neg = np.where(neg_mask & (tmp > 0), tmp, 0.0)
    return float((neg.astype(np.float64) ** 2).sum())


def kernel(features, target):
    from concourse import bass_utils

    features = np.asarray(features, np.float32)
    target = np.asarray(target)
    assert features.shape == (N, C)

    if "nc" not in _CACHE:
        _CACHE["nc"] = _build_bass()
    nc = _CACHE["nc"]

    in_maps = _make_in_maps(features, target)
    res = bass_utils.run_bass_kernel_spmd(nc, in_maps, core_ids=list(range(8)))

    tg = target.astype(np.int64)
    f64 = features.astype(np.float64)
    sq = np.einsum("ij,ij->i", f64, f64)

    # pos from the class-sum identity (S1 partials from the device)
    S1 = np.zeros((100, 512), np.float64)
    for core_out in res.results:
        S1 += np.asarray(core_out["s1"], np.float64)
    cnt = np.bincount(tg, minlength=NCLS).astype(np.float64)
    S2 = np.zeros(NCLS, np.float64)
    np.add.at(S2, tg, sq)
    pos = float((2.0 * cnt * S2).sum() - 2.0 * (S1 * S1).sum())

    # distance certificate: per [128,1024] double-tile (256 a-rows),
    #   min d >= minsq(row half-block) + minsq(col block) - 2 (maxG + err)
    half_min = sq.reshape(32, 256).min(1)
    blk_min = sq.reshape(NB, BS).min(1)
    err = FP8_RELERR * sq.max()
    ok = True
    for k, pairs in enumerate(_pair_lists()):
        mxv = np.asarray(res.results[k]["mx"], np.float32).max(axis=0)
        accv = np.asarray(res.results[k]["acc"], np.float64).sum(axis=0)
        for t, (a, b) in enumerate(pairs):
            for half in range(2):
                du = t * 2 + half
                eng = _red_engine(du)
                if eng == "A":
                    g = THRESH if accv[du] == 0.0 else np.inf
                else:
                    g = float(mxv[du])
                bound = half_min[a * 2 + half] + blk_min[b] - 2.0 * (g + err)
                if bound < MARGIN * MARGIN:
                    ok = False
    neg = 0.0 if ok else _host_neg_term(features, target)
    if os.environ.get("KERNEL_DEBUG"):
        print(f"[kernel] certificate ok={ok} pos={pos:.6e} neg={neg:.6e}")

    t = N * (N - 1)
    return np.asarray((pos + neg) / (2.0 * t), dtype=np.float32)
